# revision 3
# baseline (speedup 1.0000x reference)
"""GroupedQueryAttention Trainium2 Bass kernel — v12 (tensor-parallel heads).

Sharding: 8 cores = (batch b in {0,1}) x (kv-group g in {0..3}).  Core (b,g)
projects Q for its 4 q-heads, K/V for its single kv head (no redundant K/V
work), runs exact-causal attention over the full T=2048 rows, and computes
the PARTIAL output projection ao_g @ wo[:, cols_g].T in bf16.  The host sums
the 4 group partials per batch — no collective, no cross-core arithmetic on
device.  Per-core PE work ~20 GF vs ~25 GF for the v4 row-sharded kernel.

Engine flow: chunk tq's projections (K, V as V.T via wide-moving matmuls +
PE transposes, Q) are emitted BEFORE chunk tq-1's attention; the rope
multiplies are split out and emitted AFTER that attention block so their
swap-DMA wait never head-of-line blocks the in-order DVE queue.  The
attention inner loop is software-pipelined (score MM of s-tile si+5 issues
before the denominator/AV MMs of si) with the DIAGONAL s-tile at position 3,
so neither the scalar-engine exp latency nor the mask-add stalls the
in-order PE queue; each tile's out-projection is deferred TWO tiles and
spread through the next tile's score loop.  The softmax denominator
accumulates on the PE via an all-ones bf16 stationary.  DMA queues: x on
sync; small early constants + rope swaps + output on gpsimd; big constants
fire dependency-free from the scalar queue at rep start.  PSUM banks:
proj 2 + scores 2 + AV 1 + denominator 1 + out-proj 2 = 8.
"""

import sys

for _p in ("/opt/trn_rl_repo",):
    if _p not in sys.path:
        sys.path.insert(0, _p)

import numpy as np

B, T, D = 2, 2048, 2048
NH, NKV, HD = 16, 4, 128
NREP = NH // NKV          # 4 q-heads per kv head
P = 128
ND = D // P               # 16 contraction chunks
TCH = 512                 # t-chunk for projections
NTC = T // TCH            # 4
NTT = T // P              # 16 sequence tiles
QW = NREP * P             # 512: moving width of attention matmuls
SCALE = float(HD) ** -0.5
THETA = 10000.0
NEG = -1.0e30
CORES = list(range(8))

_prog_cache = {}


def _build_program(reps=1, no_out=False, no_attn=False):
    from contextlib import ExitStack
    from concourse import mybir, tile, bacc

    f32 = mybir.dt.float32
    bf16 = mybir.dt.bfloat16
    nc = bacc.Bacc("TRN2", target_bir_lowering=False, debug=False,
                   enable_asserts=True, num_devices=8)

    # host-prearranged inputs: every tile loads as one contiguous block
    xP = nc.dram_tensor("xP", [NTC, P, ND * TCH], bf16, kind="ExternalInput").ap()
    wqP = nc.dram_tensor("wqP", [P, NREP * ND * P], bf16, kind="ExternalInput").ap()
    wkP = nc.dram_tensor("wkP", [P, ND * HD], bf16, kind="ExternalInput").ap()
    wvP = nc.dram_tensor("wvP", [P, ND * HD], bf16, kind="ExternalInput").ap()
    woP = nc.dram_tensor("woP", [P, NREP * D], bf16, kind="ExternalInput").ap()
    ropeC = nc.dram_tensor("ropeC", [HD, T], bf16, kind="ExternalInput").ap()
    ropeS = nc.dram_tensor("ropeS", [HD, T], bf16, kind="ExternalInput").ap()
    ropeCq = nc.dram_tensor("ropeCq", [HD, NTT * QW], bf16, kind="ExternalInput").ap()
    ropeSq = nc.dram_tensor("ropeSq", [HD, NTT * QW], bf16, kind="ExternalInput").ap()
    maskd = nc.dram_tensor("maskd", [P, QW], bf16, kind="ExternalInput").ap()
    ident = nc.dram_tensor("ident", [P, P], bf16, kind="ExternalInput").ap()
    out_ext = nc.dram_tensor("out", [T, D], bf16, kind="ExternalOutput").ap()

    Exp = mybir.ActivationFunctionType.Exp

    with tile.TileContext(nc) as tc, ExitStack() as es:
        perm = es.enter_context(tc.tile_pool(name="perm", bufs=1))
        kT = perm.tile([P, T], bf16, tag="kT", name="kT")
        vT = perm.tile([P, T], bf16, tag="vT", name="vT")
        v_sb = [perm.tile([P, HD], bf16, tag=f"v{ti}", name=f"v{ti}")
                for ti in range(NTT)]
        qG = perm.tile([P, NTT * QW], bf16, tag="qG", name="qG")
        aoG = perm.tile([P, NTT * QW], bf16, tag="aoG", name="aoG")
        ones = perm.tile([P, P], bf16, tag="ones", name="ones")
        nc.vector.memset(ones, 1.0)
        if no_attn:
            nc.vector.memset(aoG, 0.0)

        cst = es.enter_context(tc.tile_pool(name="cst", bufs=1))
        wq_sb = cst.tile([P, NREP * ND * P], bf16, tag="wq", name="wq")
        wk_sb = cst.tile([P, ND * HD], bf16, tag="wk", name="wk")
        wv_sb = cst.tile([P, ND * HD], bf16, tag="wv", name="wv")
        wo_sb = cst.tile([P, NREP * D], bf16, tag="wo", name="wo")
        rc = cst.tile([P, T], bf16, tag="ropeC", name="ropeC")
        rs = cst.tile([P, T], bf16, tag="ropeS", name="ropeS")
        rcq = cst.tile([P, NTT * QW], bf16, tag="ropeCq", name="ropeCq")
        rsq = cst.tile([P, NTT * QW], bf16, tag="ropeSq", name="ropeSq")
        mk = cst.tile([P, QW], bf16, tag="maskd", name="maskd")
        id_sb = cst.tile([P, P], bf16, tag="ident", name="ident")

        for _rep in range(reps):
            with tc.tile_pool(name="px", bufs=2) as px, \
                 tc.tile_pool(name="swk", bufs=2) as swk, \
                 tc.tile_pool(name="swq", bufs=2) as swqp, \
                 tc.tile_pool(name="aex", bufs=6) as aex, \
                 tc.tile_pool(name="aden", bufs=2) as aden, \
                 tc.tile_pool(name="p5y", bufs=3) as p5y, \
                 tc.tile_pool(name="psP", bufs=2, space="PSUM") as psP, \
                 tc.tile_pool(name="psS", bufs=2, space="PSUM") as psS, \
                 tc.tile_pool(name="psO", bufs=1, space="PSUM") as psO, \
                 tc.tile_pool(name="psD", bufs=1, space="PSUM") as psD, \
                 tc.tile_pool(name="psY", bufs=2, space="PSUM") as psY:
                # constants: ordered by first use (x goes on sync; the rest
                # on the gpsimd queue, wk first so chunk-0 K proj can start)
                nc.gpsimd.dma_start(out=wk_sb, in_=wkP)
                nc.gpsimd.dma_start(out=wv_sb, in_=wvP)
                nc.gpsimd.dma_start(out=id_sb, in_=ident)
                nc.gpsimd.dma_start(out=rc, in_=ropeC)
                nc.gpsimd.dma_start(out=rs, in_=ropeS)
                nc.scalar.dma_start(out=wq_sb, in_=wqP)
                nc.scalar.dma_start(out=rcq, in_=ropeCq)
                nc.scalar.dma_start(out=rsq, in_=ropeSq)
                nc.scalar.dma_start(out=mk, in_=maskd)
                nc.scalar.dma_start(out=wo_sb, in_=woP)

                ksw, qsw = {}, {}

                def proj_chunk(tq):
                    # ---- load x chunk (4 sub-DMAs so d=0 lands early) ----
                    xq = px.tile([P, ND * TCH], bf16, tag="xq", name="xq")
                    for qd in range(4):
                        nc.sync.dma_start(
                            out=xq[:, qd * 4 * TCH:(qd + 1) * 4 * TCH],
                            in_=xP[tq][:, qd * 4 * TCH:(qd + 1) * 4 * TCH])

                    # ---- K projection for this chunk ----
                    pk = psP.tile([P, TCH], f32, tag="proj", name="pk")
                    for d in range(ND):
                        nc.tensor.matmul(
                            pk,
                            wk_sb[:, d * HD:(d + 1) * HD],
                            xq[:, d * TCH:(d + 1) * TCH],
                            start=(d == 0), stop=(d == ND - 1))
                    kc = kT[:, tq * TCH:(tq + 1) * TCH]
                    nc.vector.tensor_copy(out=kc, in_=pk)
                    # K rope (swap halves via sb->sb DMA, then 2 mul + add)
                    sw = swk.tile([P, TCH], bf16, tag="sw", name="sw")
                    nc.gpsimd.dma_start(out=sw[0:64, :], in_=kc[64:128, :])
                    nc.gpsimd.dma_start(out=sw[64:128, :], in_=kc[0:64, :])
                    ksw[tq] = sw

                    # ---- V projection as V.T (wide moving, LDW hidden),
                    # then PE-transpose each 128-tile into [t, hd] ----
                    pv = psP.tile([P, TCH], f32, tag="proj", name="pv")
                    for d in range(ND):
                        nc.tensor.matmul(
                            pv,
                            wv_sb[:, d * HD:(d + 1) * HD],
                            xq[:, d * TCH:(d + 1) * TCH],
                            start=(d == 0), stop=(d == ND - 1))
                    vc = vT[:, tq * TCH:(tq + 1) * TCH]
                    nc.vector.tensor_copy(out=vc, in_=pv)
                    for i in range(TCH // P):
                        ti = (TCH // P) * tq + i
                        pt = psP.tile([P, 4 * HD], bf16, tag="proj", name="pt")
                        nc.tensor.transpose(
                            pt[:, 0:HD], vT[:, ti * P:(ti + 1) * P], id_sb)
                        nc.vector.tensor_copy(out=v_sb[ti], in_=pt[:, 0:HD])

                    # ---- Q projection for the 4 heads of this chunk ----
                    for h in range(NREP):
                        pq = psP.tile([P, TCH], f32, tag="proj", name="pq")
                        wqh = wq_sb[:, h * ND * P:(h + 1) * ND * P]
                        for d in range(ND):
                            nc.tensor.matmul(
                                pq,
                                wqh[:, d * P:(d + 1) * P],
                                xq[:, d * TCH:(d + 1) * TCH],
                                start=(d == 0), stop=(d == ND - 1))
                        # scatter [hd, 4*128] into (ti, h, t) order
                        nc.vector.tensor_copy(
                            out=qG.rearrange("p (k h t) -> p k h t",
                                             k=NTT, h=NREP)
                            [:, 4 * tq:4 * tq + 4, h, :],
                            in_=pq.rearrange("p (a t) -> p a t", a=4))

                    # ---- Q rope for this chunk: (ti, h, t) cols ----
                    qc = qG[:, tq * 4 * QW:(tq + 1) * 4 * QW]
                    swq = swqp.tile([P, 4 * QW], bf16, tag="swq", name="swq")
                    nc.gpsimd.dma_start(out=swq[0:64, :], in_=qc[64:128, :])
                    nc.gpsimd.dma_start(out=swq[64:128, :], in_=qc[0:64, :])
                    qsw[tq] = swq

                def rope_chunk(tq):
                    # rope multiplies, emitted AFTER the previous chunk's
                    # attention so the swap-DMA wait never head-of-line
                    # blocks the in-order DVE queue
                    kc = kT[:, tq * TCH:(tq + 1) * TCH]
                    sw = ksw.pop(tq)
                    nc.vector.tensor_mul(sw, sw, rs[:, tq * TCH:(tq + 1) * TCH])
                    nc.vector.tensor_mul(kc, kc, rc[:, tq * TCH:(tq + 1) * TCH])
                    nc.vector.tensor_add(kc, kc, sw)
                    qc = qG[:, tq * 4 * QW:(tq + 1) * 4 * QW]
                    swq = qsw.pop(tq)
                    nc.vector.tensor_mul(
                        swq, swq, rsq[:, tq * 4 * QW:(tq + 1) * 4 * QW])
                    nc.vector.tensor_mul(
                        qc, qc, rcq[:, tq * 4 * QW:(tq + 1) * 4 * QW])
                    nc.vector.tensor_add(qc, qc, swq)

                def out_proj_dg(ti, dg):
                    if no_out:
                        return
                    aot = aoG.rearrange("p (k h t) -> p k h t", k=NTT, h=NREP)
                    psy = psY.tile([P, TCH], f32, tag="y", name="y")
                    for h in range(NREP):
                        nc.tensor.matmul(
                            psy,
                            aot[:, ti, h, :],
                            wo_sb[:, h * D + dg * TCH:
                                  h * D + (dg + 1) * TCH],
                            start=(h == 0), stop=(h == NREP - 1))
                    y_sb = p5y.tile([P, TCH], bf16, tag="ysb", name="ysb")
                    nc.vector.tensor_copy(out=y_sb, in_=psy)
                    nc.gpsimd.dma_start(
                        out=out_ext[ti * P:(ti + 1) * P,
                                    dg * TCH:(dg + 1) * TCH],
                        in_=y_sb)

                def out_proj(ti):
                    for dg in range(4):
                        out_proj_dg(ti, dg)

                def attn_tile(ti, mid_emit=()):
                    mid_emit = list(mid_emit)
                    if no_attn:
                        for me in mid_emit:
                            me()
                        return
                    q_mv = qG[:, ti * QW:(ti + 1) * QW]
                    pso = psO.tile([P, QW], f32, tag="av", name="av")
                    psd = psD.tile([P, QW], f32, tag="db", name="db")
                    # diagonal s-tile at position 3: its mask-add + exp
                    # latency hides under later score MMs (drain depth 3)
                    order = list(range(ti))
                    order.insert(min(3, len(order)), ti)
                    pend = []

                    def drain(stop):
                        idx, fsi, fex = pend.pop(0)
                        nc.tensor.matmul(
                            psd, ones, fex,
                            start=(idx == 0), stop=stop,
                            skip_group_check=True)
                        nc.tensor.matmul(
                            pso, v_sb[fsi], fex,
                            start=(idx == 0), stop=stop,
                            skip_group_check=True)

                    for idx, si in enumerate(order):
                        psw = psS.tile([P, QW], f32, tag="sc", name="sc")
                        nc.tensor.matmul(
                            psw, kT[:, si * P:(si + 1) * P], q_mv,
                            start=True, stop=True)
                        if si == ti:
                            nc.vector.tensor_add(psw, psw, mk)
                        exw = aex.tile([P, QW], bf16, tag="exp", name="exp")
                        nc.scalar.activation(exw, psw, Exp, scale=SCALE)
                        pend.append((idx, si, exw))
                        if len(pend) > 4:
                            drain(stop=False)
                        if idx in (3, 6, 9, 12) and mid_emit:
                            mid_emit.pop(0)()
                    while mid_emit:
                        mid_emit.pop(0)()
                    while pend:
                        drain(stop=(len(pend) == 1))
                    rden = aden.tile([P, QW], f32, tag="rden", name="rden")
                    nc.vector.reciprocal(rden, psd)
                    nc.vector.tensor_mul(
                        aoG[:, ti * QW:(ti + 1) * QW], pso, rden)

                # chunk tq+1's projections are emitted before chunk tq's
                # attention so rope chains hide under attention PE work;
                # each tile's out-proj is deferred into the next tile
                proj_chunk(0)
                hist = []
                for tq in range(NTC):
                    rope_chunk(tq)
                    if tq + 1 < NTC:
                        proj_chunk(tq + 1)
                    for i in range(TCH // P):
                        ti = (TCH // P) * tq + i
                        if len(hist) >= 2:
                            pv_ = hist.pop(0)
                            attn_tile(ti, mid_emit=[
                                (lambda p=pv_, d=dg: out_proj_dg(p, d))
                                for dg in range(4)])
                        else:
                            attn_tile(ti)
                        hist.append(ti)
                for p in hist:
                    out_proj(p)

    nc.compile()
    return nc


def _get_program(reps=1, no_out=False, no_attn=False):
    key = (reps, no_out, no_attn)
    if key not in _prog_cache:
        _prog_cache[key] = _build_program(reps, no_out, no_attn)
    return _prog_cache[key]


def _host_inputs(x, wq, wk, wv, wo):
    import ml_dtypes
    bf16 = ml_dtypes.bfloat16

    x = np.asarray(x, dtype=np.float32)
    wq = np.asarray(wq, dtype=np.float32)
    wk = np.asarray(wk, dtype=np.float32)
    wv = np.asarray(wv, dtype=np.float32)
    wo = np.asarray(wo, dtype=np.float32)

    # de-interleave head dims: even (real) components first, odd (imag) last
    perm128 = np.concatenate([np.arange(0, HD, 2), np.arange(1, HD, 2)])

    freqs = (1.0 / THETA ** (np.arange(0, HD, 2)[: HD // 2] / HD)).astype(np.float64)
    t = np.arange(T, dtype=np.float64)
    ang = np.outer(freqs, t)                                  # (64, T)
    cos = np.cos(ang)
    sin = np.sin(ang)
    ropeC = np.concatenate([cos, cos], axis=0).astype(bf16)   # (128, T)
    ropeS = np.concatenate([-sin, sin], axis=0).astype(bf16)
    # q-rope tables in (ti, h, t) order: broadcast over the 4 heads
    ropeCq = np.ascontiguousarray(np.broadcast_to(
        ropeC.reshape(HD, NTT, 1, P), (HD, NTT, NREP, P)).reshape(HD, NTT * QW))
    ropeSq = np.ascontiguousarray(np.broadcast_to(
        ropeS.reshape(HD, NTT, 1, P), (HD, NTT, NREP, P)).reshape(HD, NTT * QW))

    sp = np.arange(P)[:, None]
    tf = np.arange(P)[None, :]
    tri = np.where(sp <= tf, 0.0, NEG).astype(np.float32)     # (128, 128)
    maskd = np.tile(tri, (1, NREP)).astype(bf16)              # (128, 512)
    ident = np.eye(P, dtype=bf16)

    # per-batch x blocks
    xPs = []
    for b in range(B):
        xb = x[b].astype(bf16)
        xPs.append(np.ascontiguousarray(
            xb.reshape(NTC, TCH, ND, P).transpose(0, 3, 2, 1)
            .reshape(NTC, P, ND * TCH)))

    # per-group weight blocks
    wqPs, wkPs, wvPs, woPs = [], [], [], []
    for g in range(NKV):
        permq = np.concatenate(
            [(4 * g + h) * HD + perm128 for h in range(NREP)])
        wq_g = wq[permq]                                      # (512, D)
        wqPs.append(np.ascontiguousarray(
            wq_g.T.reshape(ND, P, NREP, P).transpose(1, 2, 0, 3)
            .reshape(P, NREP * ND * P).astype(bf16)))
        wk_g = wk[g * HD:(g + 1) * HD][perm128]               # (128, D)
        wkPs.append(np.ascontiguousarray(
            wk_g.T.reshape(ND, P, HD).transpose(1, 0, 2)
            .reshape(P, ND * HD).astype(bf16)))
        wv_g = wv[g * HD:(g + 1) * HD]                        # (128, D) no perm
        wvPs.append(np.ascontiguousarray(
            wv_g.T.reshape(ND, P, HD).transpose(1, 0, 2)
            .reshape(P, ND * HD).astype(bf16)))
        woPs.append(np.ascontiguousarray(
            wo.T[g * 512:(g + 1) * 512].reshape(NREP, P, D)
            .transpose(1, 0, 2).reshape(P, NREP * D).astype(bf16)))

    in_maps = []
    for c in CORES:
        b, g = c // 4, c % 4
        in_maps.append({
            "xP": xPs[b],
            "wqP": wqPs[g],
            "wkP": wkPs[g],
            "wvP": wvPs[g],
            "woP": woPs[g],
            "ropeC": ropeC,
            "ropeS": ropeS,
            "ropeCq": ropeCq,
            "ropeSq": ropeSq,
            "maskd": maskd,
            "ident": ident,
        })
    return in_maps


def _run(in_maps, reps=1):
    from concourse.bass_utils import run_bass_kernel_spmd
    nc = _get_program(reps)
    return run_bass_kernel_spmd(nc, in_maps, CORES)


def kernel(x, wq, wk, wv, wo, mask):
    import time
    in_maps = _host_inputs(x, wq, wk, wv, wo)
    try:
        res = _run(in_maps, reps=1)
    except Exception:
        # a previous heavy run can leave a core wedged
        # (NRT_EXEC_UNIT_UNRECOVERABLE); one retry recovers it
        time.sleep(2.0)
        res = _run(in_maps, reps=1)
    out = np.zeros((B, T, D), dtype=np.float32)
    for c in CORES:
        b = c // 4
        out[b] += res.results[c]["out"].astype(np.float32)
    return out


# revision 4
# speedup vs baseline: 1.0869x; 1.0869x over previous
"""GroupedQueryAttention Trainium2 Bass kernel — v12 (tensor-parallel heads).

Sharding: 8 cores = (batch b in {0,1}) x (kv-group g in {0..3}).  Core (b,g)
projects Q for its 4 q-heads, K/V for its single kv head (no redundant K/V
work), runs exact-causal attention over the full T=2048 rows, and computes
the PARTIAL output projection ao_g @ wo[:, cols_g].T in bf16.  The host sums
the 4 group partials per batch — no collective, no cross-core arithmetic on
device.  Per-core PE work ~20 GF vs ~25 GF for the v4 row-sharded kernel.

Engine flow: chunk tq's projections (K, V as V.T via wide-moving matmuls +
PE transposes, Q) are emitted BEFORE chunk tq-1's attention; the rope
multiplies are split out and emitted AFTER that attention block so their
swap-DMA wait never head-of-line blocks the in-order DVE queue.  The
attention inner loop is software-pipelined (score MM of s-tile si+4 issues
before the denominator/AV MMs of si) with the DIAGONAL s-tile at position 3,
so neither the scalar-engine exp latency nor the mask-add stalls the
in-order PE queue; each tile's out-projection is deferred TWO tiles and
spread through the next tile's score loop.  The softmax denominator
accumulates on the PE via an all-ones bf16 stationary.  DMA queues: x on
sync; small early constants + rope swaps + output on gpsimd; big constants
fire dependency-free from the scalar queue at rep start.  PSUM banks:
proj 2 + scores 2 + AV 1 + denominator 1 + out-proj 2 = 8.
"""

import sys

for _p in ("/opt/trn_rl_repo",):
    if _p not in sys.path:
        sys.path.insert(0, _p)

import numpy as np

B, T, D = 2, 2048, 2048
NH, NKV, HD = 16, 4, 128
NREP = NH // NKV          # 4 q-heads per kv head
P = 128
ND = D // P               # 16 contraction chunks
TCH = 512                 # t-chunk for projections
NTC = T // TCH            # 4
NTT = T // P              # 16 sequence tiles
QW = NREP * P             # 512: moving width of attention matmuls
SCALE = float(HD) ** -0.5
THETA = 10000.0
NEG = -1.0e30
CORES = list(range(8))

_prog_cache = {}


def _build_program(reps=1, no_out=False, no_attn=False):
    from contextlib import ExitStack
    from concourse import mybir, tile, bacc

    f32 = mybir.dt.float32
    bf16 = mybir.dt.bfloat16
    nc = bacc.Bacc("TRN2", target_bir_lowering=False, debug=False,
                   enable_asserts=True, num_devices=8)

    # host-prearranged inputs: every tile loads as one contiguous block
    xP = nc.dram_tensor("xP", [NTC, P, ND * TCH], bf16, kind="ExternalInput").ap()
    wqP = nc.dram_tensor("wqP", [P, NREP * ND * P], bf16, kind="ExternalInput").ap()
    wkP = nc.dram_tensor("wkP", [P, ND * HD], bf16, kind="ExternalInput").ap()
    wvP = nc.dram_tensor("wvP", [P, ND * HD], bf16, kind="ExternalInput").ap()
    woP = nc.dram_tensor("woP", [P, NREP * D], bf16, kind="ExternalInput").ap()
    ropeC = nc.dram_tensor("ropeC", [HD, T], bf16, kind="ExternalInput").ap()
    ropeS = nc.dram_tensor("ropeS", [HD, T], bf16, kind="ExternalInput").ap()
    ropeCq = nc.dram_tensor("ropeCq", [HD, NTT * QW], bf16, kind="ExternalInput").ap()
    ropeSq = nc.dram_tensor("ropeSq", [HD, NTT * QW], bf16, kind="ExternalInput").ap()
    maskd = nc.dram_tensor("maskd", [P, QW], bf16, kind="ExternalInput").ap()
    ident = nc.dram_tensor("ident", [P, P], bf16, kind="ExternalInput").ap()
    out_ext = nc.dram_tensor("out", [T, D], bf16, kind="ExternalOutput").ap()

    Exp = mybir.ActivationFunctionType.Exp

    with tile.TileContext(nc) as tc, ExitStack() as es:
        perm = es.enter_context(tc.tile_pool(name="perm", bufs=1))
        kT = perm.tile([P, T], bf16, tag="kT", name="kT")
        vT = perm.tile([P, T], bf16, tag="vT", name="vT")
        v_sb = [perm.tile([P, HD], bf16, tag=f"v{ti}", name=f"v{ti}")
                for ti in range(NTT)]
        qG = perm.tile([P, NTT * QW], bf16, tag="qG", name="qG")
        aoG = perm.tile([P, NTT * QW], bf16, tag="aoG", name="aoG")
        ones = perm.tile([P, P], bf16, tag="ones", name="ones")
        nc.vector.memset(ones, 1.0)
        if no_attn:
            nc.vector.memset(aoG, 0.0)

        cst = es.enter_context(tc.tile_pool(name="cst", bufs=1))
        wq_sb = cst.tile([P, NREP * ND * P], bf16, tag="wq", name="wq")
        wk_sb = cst.tile([P, ND * HD], bf16, tag="wk", name="wk")
        wv_sb = cst.tile([P, ND * HD], bf16, tag="wv", name="wv")
        wo_sb = cst.tile([P, NREP * D], bf16, tag="wo", name="wo")
        rc = cst.tile([P, T], bf16, tag="ropeC", name="ropeC")
        rs = cst.tile([P, T], bf16, tag="ropeS", name="ropeS")
        rcq = cst.tile([P, NTT * QW], bf16, tag="ropeCq", name="ropeCq")
        rsq = cst.tile([P, NTT * QW], bf16, tag="ropeSq", name="ropeSq")
        mk = cst.tile([P, QW], bf16, tag="maskd", name="maskd")
        id_sb = cst.tile([P, P], bf16, tag="ident", name="ident")

        for _rep in range(reps):
            with tc.tile_pool(name="px", bufs=2) as px, \
                 tc.tile_pool(name="swk", bufs=2) as swk, \
                 tc.tile_pool(name="swq", bufs=2) as swqp, \
                 tc.tile_pool(name="aex", bufs=6) as aex, \
                 tc.tile_pool(name="aden", bufs=2) as aden, \
                 tc.tile_pool(name="p5y", bufs=3) as p5y, \
                 tc.tile_pool(name="psP", bufs=2, space="PSUM") as psP, \
                 tc.tile_pool(name="psS", bufs=2, space="PSUM") as psS, \
                 tc.tile_pool(name="psO", bufs=1, space="PSUM") as psO, \
                 tc.tile_pool(name="psD", bufs=1, space="PSUM") as psD, \
                 tc.tile_pool(name="psY", bufs=2, space="PSUM") as psY:
                # constants: ordered by first use (x goes on sync; the rest
                # on the gpsimd queue, wk first so chunk-0 K proj can start)
                nc.gpsimd.dma_start(out=wk_sb, in_=wkP)
                nc.gpsimd.dma_start(out=wv_sb, in_=wvP)
                nc.gpsimd.dma_start(out=id_sb, in_=ident)
                nc.gpsimd.dma_start(out=rc, in_=ropeC)
                nc.gpsimd.dma_start(out=rs, in_=ropeS)
                nc.scalar.dma_start(out=wq_sb, in_=wqP)
                nc.scalar.dma_start(out=rcq, in_=ropeCq)
                nc.scalar.dma_start(out=rsq, in_=ropeSq)
                nc.scalar.dma_start(out=mk, in_=maskd)
                nc.scalar.dma_start(out=wo_sb, in_=woP)

                ksw, qsw = {}, {}

                def proj_chunk(tq):
                    # ---- load x chunk (4 sub-DMAs so d=0 lands early) ----
                    xq = px.tile([P, ND * TCH], bf16, tag="xq", name="xq")
                    for qd in range(4):
                        nc.sync.dma_start(
                            out=xq[:, qd * 4 * TCH:(qd + 1) * 4 * TCH],
                            in_=xP[tq][:, qd * 4 * TCH:(qd + 1) * 4 * TCH])

                    # ---- K projection for this chunk ----
                    pk = psP.tile([P, TCH], f32, tag="proj", name="pk")
                    for d in range(ND):
                        nc.tensor.matmul(
                            pk,
                            wk_sb[:, d * HD:(d + 1) * HD],
                            xq[:, d * TCH:(d + 1) * TCH],
                            start=(d == 0), stop=(d == ND - 1))
                    kc = kT[:, tq * TCH:(tq + 1) * TCH]
                    nc.vector.tensor_copy(out=kc, in_=pk)
                    # K rope (swap halves via sb->sb DMA, then 2 mul + add)
                    sw = swk.tile([P, TCH], bf16, tag="sw", name="sw")
                    nc.gpsimd.dma_start(out=sw[0:64, :], in_=kc[64:128, :])
                    nc.gpsimd.dma_start(out=sw[64:128, :], in_=kc[0:64, :])
                    ksw[tq] = sw

                    # ---- V projection as V.T (wide moving, LDW hidden),
                    # then PE-transpose each 128-tile into [t, hd] ----
                    pv = psP.tile([P, TCH], f32, tag="proj", name="pv")
                    for d in range(ND):
                        nc.tensor.matmul(
                            pv,
                            wv_sb[:, d * HD:(d + 1) * HD],
                            xq[:, d * TCH:(d + 1) * TCH],
                            start=(d == 0), stop=(d == ND - 1))
                    vc = vT[:, tq * TCH:(tq + 1) * TCH]
                    nc.vector.tensor_copy(out=vc, in_=pv)
                    for i in range(TCH // P):
                        ti = (TCH // P) * tq + i
                        pt = psP.tile([P, 4 * HD], bf16, tag="proj", name="pt")
                        nc.tensor.transpose(
                            pt[:, 0:HD], vT[:, ti * P:(ti + 1) * P], id_sb)
                        nc.vector.tensor_copy(out=v_sb[ti], in_=pt[:, 0:HD])

                    # ---- Q projection for the 4 heads of this chunk ----
                    for h in range(NREP):
                        pq = psP.tile([P, TCH], f32, tag="proj", name="pq")
                        wqh = wq_sb[:, h * ND * P:(h + 1) * ND * P]
                        for d in range(ND):
                            nc.tensor.matmul(
                                pq,
                                wqh[:, d * P:(d + 1) * P],
                                xq[:, d * TCH:(d + 1) * TCH],
                                start=(d == 0), stop=(d == ND - 1))
                        # scatter [hd, 4*128] into (ti, h, t) order
                        nc.vector.tensor_copy(
                            out=qG.rearrange("p (k h t) -> p k h t",
                                             k=NTT, h=NREP)
                            [:, 4 * tq:4 * tq + 4, h, :],
                            in_=pq.rearrange("p (a t) -> p a t", a=4))

                    # ---- Q rope for this chunk: (ti, h, t) cols ----
                    qc = qG[:, tq * 4 * QW:(tq + 1) * 4 * QW]
                    swq = swqp.tile([P, 4 * QW], bf16, tag="swq", name="swq")
                    nc.gpsimd.dma_start(out=swq[0:64, :], in_=qc[64:128, :])
                    nc.gpsimd.dma_start(out=swq[64:128, :], in_=qc[0:64, :])
                    qsw[tq] = swq

                def rope_chunk(tq):
                    # rope multiplies, emitted AFTER the previous chunk's
                    # attention so the swap-DMA wait never head-of-line
                    # blocks the in-order DVE queue
                    kc = kT[:, tq * TCH:(tq + 1) * TCH]
                    sw = ksw.pop(tq)
                    nc.vector.tensor_mul(sw, sw, rs[:, tq * TCH:(tq + 1) * TCH])
                    nc.vector.tensor_mul(kc, kc, rc[:, tq * TCH:(tq + 1) * TCH])
                    nc.vector.tensor_add(kc, kc, sw)
                    qc = qG[:, tq * 4 * QW:(tq + 1) * 4 * QW]
                    swq = qsw.pop(tq)
                    nc.vector.tensor_mul(
                        swq, swq, rsq[:, tq * 4 * QW:(tq + 1) * 4 * QW])
                    nc.vector.tensor_mul(
                        qc, qc, rcq[:, tq * 4 * QW:(tq + 1) * 4 * QW])
                    nc.vector.tensor_add(qc, qc, swq)

                def out_proj_dg(ti, dg):
                    if no_out:
                        return
                    aot = aoG.rearrange("p (k h t) -> p k h t", k=NTT, h=NREP)
                    psy = psY.tile([P, TCH], f32, tag="y", name="y")
                    for h in range(NREP):
                        nc.tensor.matmul(
                            psy,
                            aot[:, ti, h, :],
                            wo_sb[:, h * D + dg * TCH:
                                  h * D + (dg + 1) * TCH],
                            start=(h == 0), stop=(h == NREP - 1))
                    y_sb = p5y.tile([P, TCH], bf16, tag="ysb", name="ysb")
                    nc.vector.tensor_copy(out=y_sb, in_=psy)
                    nc.gpsimd.dma_start(
                        out=out_ext[ti * P:(ti + 1) * P,
                                    dg * TCH:(dg + 1) * TCH],
                        in_=y_sb)

                def out_proj(ti):
                    for dg in range(4):
                        out_proj_dg(ti, dg)

                def attn_tile(ti, mid_emit=()):
                    mid_emit = list(mid_emit)
                    if no_attn:
                        for me in mid_emit:
                            me()
                        return
                    q_mv = qG[:, ti * QW:(ti + 1) * QW]
                    pso = psO.tile([P, QW], f32, tag="av", name="av")
                    psd = psD.tile([P, QW], f32, tag="db", name="db")
                    # diagonal s-tile at position 3: its mask-add + exp
                    # latency hides under later score MMs (drain depth 3)
                    order = list(range(ti))
                    order.insert(min(3, len(order)), ti)
                    pend = []

                    def drain(stop):
                        idx, fsi, fex = pend.pop(0)
                        nc.tensor.matmul(
                            psd, ones, fex,
                            start=(idx == 0), stop=stop,
                            skip_group_check=True)
                        nc.tensor.matmul(
                            pso, v_sb[fsi], fex,
                            start=(idx == 0), stop=stop,
                            skip_group_check=True)

                    for idx, si in enumerate(order):
                        psw = psS.tile([P, QW], f32, tag="sc", name="sc")
                        nc.tensor.matmul(
                            psw, kT[:, si * P:(si + 1) * P], q_mv,
                            start=True, stop=True)
                        if si == ti:
                            nc.vector.tensor_add(psw, psw, mk)
                        exw = aex.tile([P, QW], bf16, tag="exp", name="exp")
                        nc.scalar.activation(exw, psw, Exp, scale=SCALE)
                        pend.append((idx, si, exw))
                        if len(pend) > 4:
                            drain(stop=False)
                        if idx in (3, 6, 9, 12) and mid_emit:
                            mid_emit.pop(0)()
                    while mid_emit:
                        mid_emit.pop(0)()
                    while pend:
                        drain(stop=(len(pend) == 1))
                    rden = aden.tile([P, QW], f32, tag="rden", name="rden")
                    nc.vector.reciprocal(rden, psd)
                    nc.vector.tensor_mul(
                        aoG[:, ti * QW:(ti + 1) * QW], pso, rden)

                # chunk tq+1's projections are emitted before chunk tq's
                # attention so rope chains hide under attention PE work;
                # each tile's out-proj is deferred into the next tile
                proj_chunk(0)
                hist = []
                for tq in range(NTC):
                    rope_chunk(tq)
                    if tq + 1 < NTC:
                        proj_chunk(tq + 1)
                    for i in range(TCH // P):
                        ti = (TCH // P) * tq + i
                        if len(hist) >= 2:
                            pv_ = hist.pop(0)
                            attn_tile(ti, mid_emit=[
                                (lambda p=pv_, d=dg: out_proj_dg(p, d))
                                for dg in range(4)])
                        else:
                            attn_tile(ti)
                        hist.append(ti)
                for p in hist:
                    out_proj(p)

    nc.compile()
    return nc


def _get_program(reps=1, no_out=False, no_attn=False):
    key = (reps, no_out, no_attn)
    if key not in _prog_cache:
        _prog_cache[key] = _build_program(reps, no_out, no_attn)
    return _prog_cache[key]


def _host_inputs(x, wq, wk, wv, wo):
    import ml_dtypes
    bf16 = ml_dtypes.bfloat16

    x = np.asarray(x, dtype=np.float32)
    wq = np.asarray(wq, dtype=np.float32)
    wk = np.asarray(wk, dtype=np.float32)
    wv = np.asarray(wv, dtype=np.float32)
    wo = np.asarray(wo, dtype=np.float32)

    # de-interleave head dims: even (real) components first, odd (imag) last
    perm128 = np.concatenate([np.arange(0, HD, 2), np.arange(1, HD, 2)])

    freqs = (1.0 / THETA ** (np.arange(0, HD, 2)[: HD // 2] / HD)).astype(np.float64)
    t = np.arange(T, dtype=np.float64)
    ang = np.outer(freqs, t)                                  # (64, T)
    cos = np.cos(ang)
    sin = np.sin(ang)
    ropeC = np.concatenate([cos, cos], axis=0).astype(bf16)   # (128, T)
    ropeS = np.concatenate([-sin, sin], axis=0).astype(bf16)
    # q-rope tables in (ti, h, t) order: broadcast over the 4 heads
    ropeCq = np.ascontiguousarray(np.broadcast_to(
        ropeC.reshape(HD, NTT, 1, P), (HD, NTT, NREP, P)).reshape(HD, NTT * QW))
    ropeSq = np.ascontiguousarray(np.broadcast_to(
        ropeS.reshape(HD, NTT, 1, P), (HD, NTT, NREP, P)).reshape(HD, NTT * QW))

    sp = np.arange(P)[:, None]
    tf = np.arange(P)[None, :]
    tri = np.where(sp <= tf, 0.0, NEG).astype(np.float32)     # (128, 128)
    maskd = np.tile(tri, (1, NREP)).astype(bf16)              # (128, 512)
    ident = np.eye(P, dtype=bf16)

    # per-batch x blocks
    xPs = []
    for b in range(B):
        xb = x[b].astype(bf16)
        xPs.append(np.ascontiguousarray(
            xb.reshape(NTC, TCH, ND, P).transpose(0, 3, 2, 1)
            .reshape(NTC, P, ND * TCH)))

    # per-group weight blocks
    wqPs, wkPs, wvPs, woPs = [], [], [], []
    for g in range(NKV):
        permq = np.concatenate(
            [(4 * g + h) * HD + perm128 for h in range(NREP)])
        wq_g = wq[permq]                                      # (512, D)
        wqPs.append(np.ascontiguousarray(
            wq_g.T.reshape(ND, P, NREP, P).transpose(1, 2, 0, 3)
            .reshape(P, NREP * ND * P).astype(bf16)))
        wk_g = wk[g * HD:(g + 1) * HD][perm128]               # (128, D)
        wkPs.append(np.ascontiguousarray(
            wk_g.T.reshape(ND, P, HD).transpose(1, 0, 2)
            .reshape(P, ND * HD).astype(bf16)))
        wv_g = wv[g * HD:(g + 1) * HD]                        # (128, D) no perm
        wvPs.append(np.ascontiguousarray(
            wv_g.T.reshape(ND, P, HD).transpose(1, 0, 2)
            .reshape(P, ND * HD).astype(bf16)))
        woPs.append(np.ascontiguousarray(
            wo.T[g * 512:(g + 1) * 512].reshape(NREP, P, D)
            .transpose(1, 0, 2).reshape(P, NREP * D).astype(bf16)))

    in_maps = []
    for c in CORES:
        b, g = c // 4, c % 4
        in_maps.append({
            "xP": xPs[b],
            "wqP": wqPs[g],
            "wkP": wkPs[g],
            "wvP": wvPs[g],
            "woP": woPs[g],
            "ropeC": ropeC,
            "ropeS": ropeS,
            "ropeCq": ropeCq,
            "ropeSq": ropeSq,
            "maskd": maskd,
            "ident": ident,
        })
    return in_maps


def _run(in_maps, reps=1):
    from concourse.bass_utils import run_bass_kernel_spmd
    nc = _get_program(reps)
    return run_bass_kernel_spmd(nc, in_maps, CORES)


def kernel(x, wq, wk, wv, wo, mask):
    import time
    in_maps = _host_inputs(x, wq, wk, wv, wo)
    try:
        res = _run(in_maps, reps=1)
    except Exception:
        # a previous heavy run can leave a core wedged
        # (NRT_EXEC_UNIT_UNRECOVERABLE); one retry recovers it
        time.sleep(2.0)
        res = _run(in_maps, reps=1)
    out = np.zeros((B, T, D), dtype=np.float32)
    for c in CORES:
        b = c // 4
        out[b] += res.results[c]["out"].astype(np.float32)
    return out
